# revision 22
# baseline (speedup 1.0000x reference)
"""Trainium2 Bass kernel for nn_AttnDecoderRNN (B=2048, L=256, H=512, O=1).

Data-parallel over batch across 8 NeuronCores (256 batch elements / core).

Math refactor (exact algebra; host folds *weight-only* constants):
  embedded = x @ emb_W + emb_b  is affine in the scalar x[b], so after
  InstanceNorm over H:
      embedded_norm[b, h] = Wc[h] * u[b] + bc[h] * v[b]
  with Wc = emb_W - mean(emb_W), bc = emb_b - mean(emb_b),
       v[b] = rsqrt(a2*x^2 + a1*x + a0 + eps), u[b] = x[b] * v[b]
  (a2 = var(emb_W), a1 = 2*cov(emb_W, emb_b), a0 = var(emb_b)).
  The embedded-half of the attn / comb matmuls therefore folds into
  K=2 matmuls with host-precomputed  [Wc;bc] @ W  matrices.

Softmax normalization is folded: the attention apply runs with
unnormalized exp(logits); comb gets  (c12 @ (uv*denom) + attnT_unnorm
@ comb_W2) * (1/denom)  which equals the normalized result.

Device layout is "transposed" ([feature_partition, batch_free]) so every
matmul keeps its contraction dim on partitions. The attention apply
(per-batch matvec over encoder_outputs, the 1 GiB stream that dominates)
runs on the PE: encoder tile [l, d] as the (fp32r) stationary operand,
exp(logits) column [l, 1] as moving, output lands at attnT[:, dt, b]
(free-dim placement, unconstrained).
"""

import sys

if "/opt/trn_rl_repo" not in sys.path:
    sys.path.insert(0, "/opt/trn_rl_repo")

import numpy as np

import concourse.bass as bass
import concourse.mybir as mybir
import concourse.tile as tile

F32 = mybir.dt.float32
F32R = mybir.dt.float32r
BF16 = mybir.dt.bfloat16
F16 = mybir.dt.float16
AF = mybir.ActivationFunctionType
ALU = mybir.AluOpType

B, L, H = 2048, 256, 512
NCORES = 8
BC = B // NCORES  # 256 batch per core
EPS = 1e-5
LT = L // 128  # 2 l-tiles
HT = H // 128  # 4 h-tiles


def build_program(Bc=BC, groups=None, cb=8):
    """Build the per-core Bass program. Bc: batch per core, groups: batch
    group sizes (asymmetric: early groups big so their GRU phase hides under
    the next group's encoder stream; last group small to shrink the exposed
    tail), cb: batch elems per encoder DMA chunk."""
    if groups is None:
        groups = (128, 96, 32) if Bc == 256 else (Bc,)
    assert sum(groups) == Bc
    for gsz in groups:
        assert gsz % cb == 0 and gsz % 32 == 0 or gsz == Bc
    n_btiles = (Bc + 127) // 128

    nc = bass.Bass()

    # ---- DRAM I/O ----
    x_d = nc.dram_tensor("x", [Bc], F32, kind="ExternalInput")
    hid_d = nc.dram_tensor("hid", [Bc, H], F32, kind="ExternalInput")
    enc_d = nc.dram_tensor("enc", [Bc, L, H], F32, kind="ExternalInput")
    wattn_d = nc.dram_tensor("wattn", [H, L], F32, kind="ExternalInput")
    wcomb_d = nc.dram_tensor("wcomb", [H, H], F16, kind="ExternalInput")
    wih_d = nc.dram_tensor("wih", [H, 3 * H], F16, kind="ExternalInput")
    whh_d = nc.dram_tensor("whh", [H, 3 * H], F16, kind="ExternalInput")
    smalls_d = nc.dram_tensor("smalls", [128, 928], F32, kind="ExternalInput")
    ident_d = nc.dram_tensor("ident", [128, 128], F32, kind="ExternalInput")

    oy_d = nc.dram_tensor("out_y", [Bc], F32, kind="ExternalOutput")
    oh_d = nc.dram_tensor("out_h", [Bc, H], F32, kind="ExternalOutput")
    oaw_d = nc.dram_tensor("out_aw", [Bc, L], F32, kind="ExternalOutput")

    with tile.TileContext(nc) as tc:
        with (
            tc.tile_pool(name="consts", bufs=1) as consts,
            tc.tile_pool(name="acts", bufs=1) as acts,
            tc.tile_pool(name="epool", bufs=12) as epool,
            tc.tile_pool(name="grp", bufs=2) as grp,
            tc.tile_pool(name="ew", bufs=16) as ew,
            tc.tile_pool(name="psa", bufs=2, space="PSUM") as psa,
            tc.tile_pool(name="psw", bufs=6, space="PSUM") as psw,
        ):
            # ---- constants into SBUF ----
            # critical-path DMAs first, spread over both HWDGE queues
            ident = consts.tile([128, 128], F32)
            nc.sync.dma_start(out=ident, in_=ident_d[:, :])
            smalls = consts.tile([128, 928], F32)
            nc.scalar.dma_start(out=smalls, in_=smalls_d[:, :])
            w_attn = consts.tile([128, HT, L], F32)
            nc.scalar.dma_start(
                out=w_attn, in_=wattn_d[:, :].rearrange("(t p) m -> p t m", p=128)
            )
            # views into the packed small-constant image (see host_constants)
            attnb = smalls[:, 0:2]
            combb = smalls[:, 2:6]
            brz = smalls[:, 6:14]
            bihn = smalls[:, 14:18]
            bhhn = smalls[:, 18:22]
            outw = smalls[:, 22:26]
            ones = smalls[:, 26:27]
            p12 = smalls[0:2, 27:283]
            c12 = smalls[0:2, 283:795]
            coef = smalls[0:1, 795:798]
            outb = smalls[0:1, 798:799]
            ones_row = smalls[0:1, 800:928]
            nc.vector.memset(smalls[:, 26:27], 1.0)
            nc.vector.memset(smalls[0:1, 800:928], 1.0)

            # ---- x row -> u, v ----
            x_s = acts.tile([1, Bc], F32)
            nc.sync.dma_start(out=x_s, in_=x_d[:].rearrange("(p f) -> p f", p=1))
            xsq = acts.tile([1, Bc], F32)
            nc.vector.tensor_mul(xsq, x_s, x_s)
            var = acts.tile([1, Bc], F32)
            nc.vector.tensor_scalar_mul(var, xsq, coef[0:1, 0:1])
            t2 = acts.tile([1, Bc], F32)
            nc.vector.tensor_scalar_mul(t2, x_s, coef[0:1, 1:2])
            nc.vector.tensor_add(var, var, t2)
            nc.vector.tensor_scalar_add(var, var, coef[0:1, 2:3])
            sq = acts.tile([1, Bc], F32)
            nc.scalar.activation(sq, var, AF.Sqrt)
            v_row = acts.tile([1, Bc], F32)
            nc.vector.reciprocal(v_row, sq)
            u_row = acts.tile([1, Bc], F32)
            nc.vector.tensor_mul(u_row, x_s, v_row)
            uv = acts.tile([2, Bc], F32)
            nc.sync.dma_start(out=uv[0:1, :], in_=u_row)
            nc.sync.dma_start(out=uv[1:2, :], in_=v_row)

            # ---- hidden^T via PE transposes ----
            hidT = acts.tile([128, HT, Bc], F32)
            hidT_r = acts.tile([128, HT, Bc], F16)
            for bt in range(n_btiles):
                pb = min(128, Bc - bt * 128)
                hnat = grp.tile([128, H], F32, tag="hnat")
                nc.sync.dma_start(
                    out=hnat[:pb, :], in_=hid_d[bt * 128 : bt * 128 + pb, :]
                )
                for ht in range(HT):
                    tp = psw.tile([128, 128], F32, tag="w")
                    nc.tensor.transpose(
                        tp[:, :pb],
                        hnat[:pb, ht * 128 : (ht + 1) * 128],
                        ident[:pb, :pb],
                    )
                    nc.vector.tensor_copy(
                        hidT[:, ht, bt * 128 : bt * 128 + pb], tp[:, :pb]
                    )
                    nc.scalar.copy(
                        hidT_r[:, ht, bt * 128 : bt * 128 + pb], tp[:, :pb]
                    )

            # ---- attn logits^T -> exp (unnormalized attention weights) ----
            wexpT = acts.tile([128, LT, Bc], F32)
            wexpH = acts.tile([128, LT, Bc], F16)
            for lt in range(LT):
                lg = psw.tile([128, Bc], F32, tag="w")
                for kt in range(HT):
                    nc.tensor.matmul(
                        lg,
                        w_attn[:, kt, lt * 128 : (lt + 1) * 128],
                        hidT[:, kt, :],
                        start=(kt == 0),
                        stop=False,
                    )
                nc.tensor.matmul(
                    lg,
                    p12[:, lt * 128 : (lt + 1) * 128],
                    uv,
                    start=False,
                    stop=True,
                )
                nc.scalar.activation(
                    wexpT[:, lt, :], lg, AF.Exp, bias=attnb[:, lt : lt + 1]
                )
                nc.vector.tensor_copy(wexpH[:, lt, :], wexpT[:, lt, :])

            # ---- softmax denominators: column & row forms ----
            recipc = acts.tile([128, n_btiles], F32)  # 1/denom, [b_part, bt]
            denc = acts.tile([128, n_btiles], F32)  # denom,   [b_part, bt]
            recipr = acts.tile([1, Bc], F32)  # 1/denom row
            denr = acts.tile([1, Bc], F32)  # denom row
            for bt in range(n_btiles):
                pb = min(128, Bc - bt * 128)
                dn = psw.tile([128, 1], F32, tag="w")
                for lt in range(LT):
                    nc.tensor.matmul(
                        dn[:pb, :],
                        wexpT[:, lt, bt * 128 : bt * 128 + pb],
                        ones,
                        start=(lt == 0),
                        stop=(lt == LT - 1),
                    )
                nc.vector.reciprocal(recipc[:pb, bt : bt + 1], dn[:pb, :])
                nc.vector.tensor_copy(denc[:pb, bt : bt + 1], dn[:pb, :])
                for src, dstrow in ((recipc, recipr), (denc, denr)):
                    tpr = psw.tile([1, 128], F32, tag="w")
                    nc.tensor.transpose(
                        tpr[:, :pb], src[:pb, bt : bt + 1], ident[:pb, :pb]
                    )
                    nc.vector.tensor_copy(
                        dstrow[:, bt * 128 : bt * 128 + pb], tpr[:, :pb]
                    )

            # uv2 = uv * denom (for the comb embedded-part normalization fold)
            duv = acts.tile([2, Bc], F32)
            nc.sync.dma_start(out=duv[0:1, :], in_=denr)
            nc.sync.dma_start(out=duv[1:2, :], in_=denr)
            uv2 = acts.tile([2, Bc], F32)
            nc.vector.tensor_mul(uv2, uv, duv)

            # heavy weights, needed only from the comb/GRU phase on;
            # emitted late so the attention-critical path wins scheduler
            # priority and the encoder stream starts immediately
            w_comb = consts.tile([128, HT, H], F16)
            nc.scalar.dma_start(
                out=w_comb, in_=wcomb_d[:, :].rearrange("(t p) m -> p t m", p=128)
            )
            w_ih = consts.tile([128, HT, 3 * H], F16)
            nc.scalar.dma_start(
                out=w_ih, in_=wih_d[:, :].rearrange("(t p) m -> p t m", p=128)
            )
            w_hh = consts.tile([128, HT, 3 * H], F16)
            nc.scalar.dma_start(
                out=w_hh, in_=whh_d[:, :].rearrange("(t p) m -> p t m", p=128)
            )

            # ---- main loop over batch groups ----
            b0 = 0
            for g, gb in enumerate(groups):
                n_chunks = gb // cb
                bt = b0 // 128

                # 1/denom broadcast to all partitions, [128, gb]
                rbp = psw.tile([128, gb], F32, tag="w")
                nc.tensor.matmul(
                    rbp, ones_row, recipr[0:1, b0 : b0 + gb], start=True, stop=True
                )
                rb = grp.tile([128, gb], F32, tag="rb")
                nc.vector.tensor_copy(rb, rbp)

                # attention apply: attnT[:, dt, b] += E[b,lt]^T[:,dt] @ wexp[b,lt]
                psAT = psa.tile([128, HT, gb], F32, tag="attn")
                for c in range(n_chunks):
                    cb0 = b0 + c * cb
                    et0 = epool.tile([128, cb, H], F16, tag="E")
                    et1 = epool.tile([128, cb, H], F16, tag="E")
                    nc.gpsimd.dma_start(
                        out=et0,
                        in_=enc_d[cb0 : cb0 + cb, 0:128, :].rearrange(
                            "b l d -> l b d"
                        ),
                    )
                    nc.gpsimd.dma_start(
                        out=et1,
                        in_=enc_d[cb0 : cb0 + cb, 128:256, :].rearrange(
                            "b l d -> l b d"
                        ),
                    )
                    for bi in range(cb):
                        b = cb0 + bi
                        bl = b - b0
                        for dt in range(HT):
                            nc.tensor.matmul(
                                psAT[:, dt, bl : bl + 1],
                                et0[:, bi, dt * 128 : (dt + 1) * 128],
                                wexpH[:, 0, b : b + 1],
                                start=True,
                                stop=False,
                            )
                            nc.tensor.matmul(
                                psAT[:, dt, bl : bl + 1],
                                et1[:, bi, dt * 128 : (dt + 1) * 128],
                                wexpH[:, 1, b : b + 1],
                                start=False,
                                stop=True,
                            )

                attnT = grp.tile([128, HT, gb], F16, tag="attnT")
                nc.vector.tensor_copy(attnT, psAT)

                # comb + relu -> reluT [h', b]
                reluT = grp.tile([128, HT, gb], F16, tag="reluT")
                for mt in range(HT):
                    cps = psw.tile([128, gb], F32, tag="w")
                    nc.tensor.matmul(
                        cps,
                        c12[:, mt * 128 : (mt + 1) * 128],
                        uv2[:, b0 : b0 + gb],
                        start=True,
                        stop=False,
                    )
                    for kt in range(HT):
                        nc.tensor.matmul(
                            cps,
                            w_comb[:, kt, mt * 128 : (mt + 1) * 128],
                            attnT[:, kt, :],
                            start=False,
                            stop=(kt == HT - 1),
                        )
                    # psum * (1/denom broadcast), then relu(. + comb_b)
                    stmp = ew.tile([128, gb], F32, tag="e")
                    nc.vector.tensor_mul(stmp, cps, rb)
                    nc.scalar.activation(
                        reluT[:, mt, :], stmp, AF.Relu, bias=combb[:, mt : mt + 1]
                    )

                # GRU gate matmuls
                rza = psw.tile([128, 512], F32, tag="w")
                rzb = psw.tile([128, 512], F32, tag="w")
                gin = psw.tile([128, 512], F32, tag="w")
                ghn = psw.tile([128, 512], F32, tag="w")
                for j in range(HT):
                    for dst, col0 in ((rza, j * 128), (rzb, 512 + j * 128)):
                        for kt in range(HT):
                            nc.tensor.matmul(
                                dst[:, j * 128 : j * 128 + gb],
                                w_ih[:, kt, col0 : col0 + 128],
                                reluT[:, kt, :],
                                start=(kt == 0),
                                stop=False,
                            )
                        for kt in range(HT):
                            nc.tensor.matmul(
                                dst[:, j * 128 : j * 128 + gb],
                                w_hh[:, kt, col0 : col0 + 128],
                                hidT_r[:, kt, b0 : b0 + gb],
                                start=False,
                                stop=(kt == HT - 1),
                            )
                    for dst, wsrc, rhs_fn in (
                        (gin, w_ih, lambda kt: reluT[:, kt, :]),
                        (ghn, w_hh, lambda kt: hidT_r[:, kt, b0 : b0 + gb]),
                    ):
                        col0 = 1024 + j * 128
                        for kt in range(HT):
                            nc.tensor.matmul(
                                dst[:, j * 128 : j * 128 + gb],
                                wsrc[:, kt, col0 : col0 + 128],
                                rhs_fn(kt),
                                start=(kt == 0),
                                stop=(kt == HT - 1),
                            )

                # elementwise GRU + outputs
                hnat = grp.tile([128, H], F32, tag="hnewnat")
                awn = grp.tile([128, L], F32, tag="awn")
                hnew_tiles = []
                for j in range(HT):
                    jc = slice(j * 128, j * 128 + gb)
                    r_j = ew.tile([128, gb], F32, tag="e")
                    nc.scalar.activation(
                        r_j, rza[:, jc], AF.Sigmoid, bias=brz[:, j : j + 1]
                    )
                    z_j = ew.tile([128, gb], F32, tag="e")
                    nc.scalar.activation(
                        z_j, rzb[:, jc], AF.Sigmoid, bias=brz[:, HT + j : HT + j + 1]
                    )
                    s1 = ew.tile([128, gb], F32, tag="e")
                    nc.vector.scalar_tensor_tensor(
                        s1, ghn[:, jc], bhhn[:, j : j + 1], r_j,
                        op0=ALU.add, op1=ALU.mult,
                    )
                    s2 = ew.tile([128, gb], F32, tag="e")
                    nc.vector.tensor_add(s2, s1, gin[:, jc])
                    n_j = ew.tile([128, gb], F32, tag="e")
                    nc.scalar.activation(n_j, s2, AF.Tanh, bias=bihn[:, j : j + 1])
                    d_j = ew.tile([128, gb], F32, tag="e")
                    nc.vector.tensor_sub(d_j, hidT[:, j, b0 : b0 + gb], n_j)
                    m_j = ew.tile([128, gb], F32, tag="e")
                    nc.vector.tensor_mul(m_j, z_j, d_j)
                    hnew_j = ew.tile([128, gb], F32, tag="hn")
                    nc.vector.tensor_add(hnew_j, n_j, m_j)
                    hnew_tiles.append(hnew_j)
                    tp = psw.tile([128, 128], F32, tag="w")
                    nc.tensor.transpose(tp[:gb, :], hnew_j, ident[:, :])
                    nc.vector.tensor_copy(
                        hnat[:gb, j * 128 : (j + 1) * 128], tp[:gb, :]
                    )
                nc.sync.dma_start(out=oh_d[b0 : b0 + gb, :], in_=hnat[:gb, :])

                # final output scalar: dot(h_new, out_W) + out_b
                oy = psw.tile([1, gb], F32, tag="w")
                for kt in range(HT):
                    nc.tensor.matmul(
                        oy,
                        outw[:, kt : kt + 1],
                        hnew_tiles[kt],
                        start=(kt == 0),
                        stop=(kt == HT - 1),
                    )
                oys = ew.tile([1, gb], F32, tag="oys")
                nc.scalar.activation(oys, oy, AF.Identity, bias=outb[0:1, 0:1])
                nc.sync.dma_start(
                    out=oy_d[b0 : b0 + gb].rearrange("(p f) -> p f", p=1), in_=oys
                )

                # normalized attention weights, natural layout
                for lt in range(LT):
                    tp = psw.tile([128, 128], F32, tag="w")
                    nc.tensor.transpose(
                        tp[:gb, :], wexpT[:, lt, b0 : b0 + gb], ident[:, :]
                    )
                    nc.vector.tensor_scalar(
                        awn[:gb, lt * 128 : (lt + 1) * 128],
                        tp[:gb, :],
                        recipc[b0 - bt * 128 : b0 - bt * 128 + gb, bt : bt + 1],
                        None,
                        op0=ALU.mult,
                    )
                nc.sync.dma_start(out=oaw_d[b0 : b0 + gb, :], in_=awn[:gb, :])
                b0 += gb

    return nc


def host_constants(emb_W, emb_b, attn_W, attn_b, comb_W, comb_b, gru_bih, gru_bhh,
                   out_W, out_b):
    """Fold weight-only constants (float64 for accuracy, cast to f32)."""
    W = emb_W.astype(np.float64).reshape(H)
    bb = emb_b.astype(np.float64)
    Wc = W - W.mean()
    bc = bb - bb.mean()
    a2 = float((Wc * Wc).mean())
    a1 = float(2.0 * (Wc * bc).mean())
    a0 = float((bc * bc).mean())
    attn_W1 = attn_W[:H].astype(np.float64)
    attn_W2 = attn_W[H:].astype(np.float32)
    comb_W1 = comb_W[:H].astype(np.float64)
    comb_W2 = comb_W[H:].astype(np.float32)
    p12 = np.stack([Wc @ attn_W1, bc @ attn_W1]).astype(np.float32)
    c12 = np.stack([Wc @ comb_W1, bc @ comb_W1]).astype(np.float32)
    coef = np.array([a2, a1, a0 + EPS], np.float32)
    brz = (gru_bih.astype(np.float64) + gru_bhh.astype(np.float64))[: 2 * H].astype(
        np.float32
    )
    sm = np.zeros((128, 928), np.float32)
    sm[:, 0:2] = attn_b.astype(np.float32).reshape(2, 128).T
    sm[:, 2:6] = comb_b.astype(np.float32).reshape(4, 128).T
    sm[:, 6:14] = brz.reshape(8, 128).T
    sm[:, 14:18] = gru_bih[2 * H :].astype(np.float32).reshape(4, 128).T
    sm[:, 18:22] = gru_bhh[2 * H :].astype(np.float32).reshape(4, 128).T
    sm[:, 22:26] = out_W[:, 0].astype(np.float32).reshape(4, 128).T
    # cols 26, 27:155 are memset to 1.0 on device (ones columns)
    sm[0:2, 27:283] = p12
    sm[0:2, 283:795] = c12
    sm[0:1, 795:798] = coef
    sm[0, 798] = float(out_b[0])
    return dict(
        wattn=np.ascontiguousarray(attn_W2),
        wcomb=np.ascontiguousarray(comb_W2.astype(np.float16)),
        smalls=sm,
        ident=np.eye(128, dtype=np.float32),
    )


def _split_waits(nc):
    """Walrus codegen in this toolchain gives most ISA instructions a single
    sync-wait slot (DMA triggers fit two) and refuses to split multi-wait
    instructions itself. Insert same-engine NoOps, each absorbing one excess
    wait, ahead of any over-subscribed instruction."""
    for f in nc.m.functions:
        for blk in f.blocks:
            out = []
            changed = False
            for inst in blk.instructions:
                si = inst.sync_info
                limit = 1
                if si is not None and len(si.on_wait) > limit:
                    waits = list(si.on_wait)
                    for w in waits[:-limit]:
                        nop = mybir.InstNoOp(
                            name=nc.get_next_instruction_name(),
                            engine=inst.engine,
                        )
                        nop.sync_info = mybir.SyncInfo(on_wait=[w], on_update=[])
                        out.append(nop)
                    inst.sync_info = mybir.SyncInfo(
                        on_wait=waits[-limit:], on_update=list(si.on_update)
                    )
                    changed = True
                out.append(inst)
            if changed:
                blk.instructions = out
    return nc


_PROGRAM_CACHE = {}
TRACE = False  # set by test harness to capture an NTFF profile
LAST_RESULTS = None


def _get_program(key=(BC, None, 8)):
    if key not in _PROGRAM_CACHE:
        _PROGRAM_CACHE[key] = _split_waits(build_program(*key))
    return _PROGRAM_CACHE[key]


def kernel(x, hidden, encoder_outputs, emb_W, emb_b, attn_W, attn_b,
           comb_W, comb_b, gru_Wih, gru_Whh, gru_bih, gru_bhh, out_W, out_b):
    from concourse.bass_utils import run_bass_kernel_spmd

    x = np.asarray(x, np.float32)
    hidden = np.asarray(hidden, np.float32)
    encoder_outputs = np.asarray(encoder_outputs, np.float32)

    const = host_constants(
        np.asarray(emb_W), np.asarray(emb_b), np.asarray(attn_W),
        np.asarray(attn_b), np.asarray(comb_W), np.asarray(comb_b),
        np.asarray(gru_bih), np.asarray(gru_bhh), np.asarray(out_W),
        np.asarray(out_b),
    )
    const["wih"] = np.ascontiguousarray(np.asarray(gru_Wih, np.float16))
    const["whh"] = np.ascontiguousarray(np.asarray(gru_Whh, np.float16))

    xf = x.reshape(B)
    hid = hidden.reshape(B, H)

    in_maps = []
    for c in range(NCORES):
        sl = slice(c * BC, (c + 1) * BC)
        m = dict(const)
        m["x"] = np.ascontiguousarray(xf[sl])
        m["hid"] = np.ascontiguousarray(hid[sl])
        m["enc"] = np.ascontiguousarray(encoder_outputs[sl])
        in_maps.append(m)

    nc = _get_program()
    res = run_bass_kernel_spmd(
        nc, in_maps, core_ids=list(range(NCORES)), trace=TRACE
    )
    globals()["LAST_RESULTS"] = res

    out_y = np.concatenate([r["out_y"] for r in res.results]).reshape(B, 1, 1)
    out_h = np.concatenate([r["out_h"] for r in res.results]).reshape(1, B, H)
    out_aw = np.concatenate([r["out_aw"] for r in res.results]).reshape(B, 1, L)
    return out_y, out_h, out_aw


# revision 23
# speedup vs baseline: 1.2949x; 1.2949x over previous
"""Trainium2 Bass kernel for nn_AttnDecoderRNN (B=2048, L=256, H=512, O=1).

Data-parallel over batch across 8 NeuronCores (256 batch elements / core).

Math refactor (exact algebra; host folds *weight-only* constants):
  embedded = x @ emb_W + emb_b  is affine in the scalar x[b], so after
  InstanceNorm over H:
      embedded_norm[b, h] = Wc[h] * u[b] + bc[h] * v[b]
  with Wc = emb_W - mean(emb_W), bc = emb_b - mean(emb_b),
       v[b] = rsqrt(a2*x^2 + a1*x + a0 + eps), u[b] = x[b] * v[b]
  (a2 = var(emb_W), a1 = 2*cov(emb_W, emb_b), a0 = var(emb_b)).
  The embedded-half of the attn / comb matmuls therefore folds into
  K=2 matmuls with host-precomputed  [Wc;bc] @ W  matrices.

Softmax normalization is folded: the attention apply runs with
unnormalized exp(logits); comb gets  (c12 @ (uv*denom) + attnT_unnorm
@ comb_W2) * (1/denom)  which equals the normalized result.

Device layout is "transposed" ([feature_partition, batch_free]) so every
matmul keeps its contraction dim on partitions. The attention apply
(per-batch matvec over encoder_outputs, the 1 GiB stream that dominates)
runs on the PE: encoder tile [l, d] as the (fp32r) stationary operand,
exp(logits) column [l, 1] as moving, output lands at attnT[:, dt, b]
(free-dim placement, unconstrained).
"""

import sys

if "/opt/trn_rl_repo" not in sys.path:
    sys.path.insert(0, "/opt/trn_rl_repo")

import numpy as np

import concourse.bass as bass
import concourse.mybir as mybir
import concourse.tile as tile

F32 = mybir.dt.float32
F32R = mybir.dt.float32r
BF16 = mybir.dt.bfloat16
F16 = mybir.dt.float16
AF = mybir.ActivationFunctionType
ALU = mybir.AluOpType

B, L, H = 2048, 256, 512
NCORES = 8
BC = B // NCORES  # 256 batch per core
EPS = 1e-5
LT = L // 128  # 2 l-tiles
HT = H // 128  # 4 h-tiles


def build_program(Bc=BC, groups=None, cb=8):
    """Build the per-core Bass program. Bc: batch per core, groups: batch
    group sizes (asymmetric: early groups big so their GRU phase hides under
    the next group's encoder stream; last group small to shrink the exposed
    tail), cb: batch elems per encoder DMA chunk."""
    if groups is None:
        groups = (128, 96, 32) if Bc == 256 else (Bc,)
    assert sum(groups) == Bc
    for gsz in groups:
        assert gsz % 32 == 0 or gsz == Bc
    n_btiles = (Bc + 127) // 128

    nc = bass.Bass()

    # ---- DRAM I/O ----
    x_d = nc.dram_tensor("x", [Bc], F32, kind="ExternalInput")
    hid_d = nc.dram_tensor("hid", [Bc, H], F32, kind="ExternalInput")
    enc_d = nc.dram_tensor("enc", [Bc, L, H], F32, kind="ExternalInput")
    wattn_d = nc.dram_tensor("wattn", [H, L], F32, kind="ExternalInput")
    wcomb_d = nc.dram_tensor("wcomb", [H, H], F16, kind="ExternalInput")
    wih_d = nc.dram_tensor("wih", [H, 3 * H], F16, kind="ExternalInput")
    whh_d = nc.dram_tensor("whh", [H, 3 * H], F16, kind="ExternalInput")
    smalls_d = nc.dram_tensor("smalls", [128, 928], F32, kind="ExternalInput")
    ident_d = nc.dram_tensor("ident", [128, 128], F32, kind="ExternalInput")

    oy_d = nc.dram_tensor("out_y", [Bc], F32, kind="ExternalOutput")
    oh_d = nc.dram_tensor("out_h", [Bc, H], F32, kind="ExternalOutput")
    oaw_d = nc.dram_tensor("out_aw", [Bc, L], F32, kind="ExternalOutput")

    with tile.TileContext(nc) as tc:
        with (
            tc.tile_pool(name="consts", bufs=1) as consts,
            tc.tile_pool(name="acts", bufs=1) as acts,
            tc.tile_pool(name="epool", bufs=13) as epool,
            tc.tile_pool(name="grp", bufs=2) as grp,
            tc.tile_pool(name="ew", bufs=16) as ew,
            tc.tile_pool(name="psa", bufs=2, space="PSUM") as psa,
            tc.tile_pool(name="psw", bufs=6, space="PSUM") as psw,
        ):
            # ---- constants into SBUF ----
            # critical-path DMAs first, spread over both HWDGE queues
            ident = consts.tile([128, 128], F32)
            nc.sync.dma_start(out=ident, in_=ident_d[:, :])
            smalls = consts.tile([128, 928], F32)
            nc.scalar.dma_start(out=smalls, in_=smalls_d[:, :])
            w_attn = consts.tile([128, HT, L], F32)
            nc.scalar.dma_start(
                out=w_attn, in_=wattn_d[:, :].rearrange("(t p) m -> p t m", p=128)
            )
            # views into the packed small-constant image (see host_constants)
            attnb = smalls[:, 0:2]
            combb = smalls[:, 2:6]
            brz = smalls[:, 6:14]
            bihn = smalls[:, 14:18]
            bhhn = smalls[:, 18:22]
            outw = smalls[:, 22:26]
            ones = smalls[:, 26:27]
            p12 = smalls[0:2, 27:283]
            c12 = smalls[0:2, 283:795]
            coef = smalls[0:1, 795:798]
            outb = smalls[0:1, 798:799]
            ones_row = smalls[0:1, 800:928]
            nc.vector.memset(smalls[:, 26:27], 1.0)
            nc.vector.memset(smalls[0:1, 800:928], 1.0)

            # ---- x row -> u, v ----
            x_s = acts.tile([1, Bc], F32)
            nc.sync.dma_start(out=x_s, in_=x_d[:].rearrange("(p f) -> p f", p=1))
            xsq = acts.tile([1, Bc], F32)
            nc.vector.tensor_mul(xsq, x_s, x_s)
            var = acts.tile([1, Bc], F32)
            nc.vector.tensor_scalar_mul(var, xsq, coef[0:1, 0:1])
            t2 = acts.tile([1, Bc], F32)
            nc.vector.tensor_scalar_mul(t2, x_s, coef[0:1, 1:2])
            nc.vector.tensor_add(var, var, t2)
            nc.vector.tensor_scalar_add(var, var, coef[0:1, 2:3])
            sq = acts.tile([1, Bc], F32)
            nc.scalar.activation(sq, var, AF.Sqrt)
            v_row = acts.tile([1, Bc], F32)
            nc.vector.reciprocal(v_row, sq)
            u_row = acts.tile([1, Bc], F32)
            nc.vector.tensor_mul(u_row, x_s, v_row)
            uv = acts.tile([2, Bc], F32)
            nc.sync.dma_start(out=uv[0:1, :], in_=u_row)
            nc.sync.dma_start(out=uv[1:2, :], in_=v_row)

            # ---- hidden^T via PE transposes ----
            hidT = acts.tile([128, HT, Bc], F32)
            hidT_r = acts.tile([128, HT, Bc], F16)
            for bt in range(n_btiles):
                pb = min(128, Bc - bt * 128)
                hnat = grp.tile([128, H], F32, tag="hnat")
                nc.sync.dma_start(
                    out=hnat[:pb, :], in_=hid_d[bt * 128 : bt * 128 + pb, :]
                )
                for ht in range(HT):
                    tp = psw.tile([128, 128], F32, tag="w")
                    nc.tensor.transpose(
                        tp[:, :pb],
                        hnat[:pb, ht * 128 : (ht + 1) * 128],
                        ident[:pb, :pb],
                    )
                    nc.vector.tensor_copy(
                        hidT[:, ht, bt * 128 : bt * 128 + pb], tp[:, :pb]
                    )
                    nc.scalar.copy(
                        hidT_r[:, ht, bt * 128 : bt * 128 + pb], tp[:, :pb]
                    )

            # ---- attn logits^T -> exp (unnormalized attention weights) ----
            wexpT = acts.tile([128, LT, Bc], F32)
            wexpH = acts.tile([128, LT, Bc], F16)
            for lt in range(LT):
                lg = psw.tile([128, Bc], F32, tag="w")
                for kt in range(HT):
                    nc.tensor.matmul(
                        lg,
                        w_attn[:, kt, lt * 128 : (lt + 1) * 128],
                        hidT[:, kt, :],
                        start=(kt == 0),
                        stop=False,
                    )
                nc.tensor.matmul(
                    lg,
                    p12[:, lt * 128 : (lt + 1) * 128],
                    uv,
                    start=False,
                    stop=True,
                )
                nc.scalar.activation(
                    wexpT[:, lt, :], lg, AF.Exp, bias=attnb[:, lt : lt + 1]
                )
                nc.vector.tensor_copy(wexpH[:, lt, :], wexpT[:, lt, :])

            # ---- softmax denominators: column & row forms ----
            recipc = acts.tile([128, n_btiles], F32)  # 1/denom, [b_part, bt]
            denc = acts.tile([128, n_btiles], F32)  # denom,   [b_part, bt]
            recipr = acts.tile([1, Bc], F32)  # 1/denom row
            denr = acts.tile([1, Bc], F32)  # denom row
            for bt in range(n_btiles):
                pb = min(128, Bc - bt * 128)
                dn = psw.tile([128, 1], F32, tag="w")
                for lt in range(LT):
                    nc.tensor.matmul(
                        dn[:pb, :],
                        wexpT[:, lt, bt * 128 : bt * 128 + pb],
                        ones,
                        start=(lt == 0),
                        stop=(lt == LT - 1),
                    )
                nc.vector.reciprocal(recipc[:pb, bt : bt + 1], dn[:pb, :])
                nc.vector.tensor_copy(denc[:pb, bt : bt + 1], dn[:pb, :])
                for src, dstrow in ((recipc, recipr), (denc, denr)):
                    tpr = psw.tile([1, 128], F32, tag="w")
                    nc.tensor.transpose(
                        tpr[:, :pb], src[:pb, bt : bt + 1], ident[:pb, :pb]
                    )
                    nc.vector.tensor_copy(
                        dstrow[:, bt * 128 : bt * 128 + pb], tpr[:, :pb]
                    )

            # uv2 = uv * denom (for the comb embedded-part normalization fold)
            duv = acts.tile([2, Bc], F32)
            nc.sync.dma_start(out=duv[0:1, :], in_=denr)
            nc.sync.dma_start(out=duv[1:2, :], in_=denr)
            uv2 = acts.tile([2, Bc], F32)
            nc.vector.tensor_mul(uv2, uv, duv)

            # heavy weights, needed only from the comb/GRU phase on;
            # emitted late so the attention-critical path wins scheduler
            # priority and the encoder stream starts immediately
            w_comb = consts.tile([128, HT, H], F16)
            nc.scalar.dma_start(
                out=w_comb, in_=wcomb_d[:, :].rearrange("(t p) m -> p t m", p=128)
            )
            w_ih = consts.tile([128, HT, 3 * H], F16)
            nc.scalar.dma_start(
                out=w_ih, in_=wih_d[:, :].rearrange("(t p) m -> p t m", p=128)
            )
            w_hh = consts.tile([128, HT, 3 * H], F16)
            nc.scalar.dma_start(
                out=w_hh, in_=whh_d[:, :].rearrange("(t p) m -> p t m", p=128)
            )

            # ---- main loop over batch groups ----
            b0 = 0
            for g, gb in enumerate(groups):
                gcb = cb if gb > 32 else min(cb, 4)
                n_chunks = gb // gcb
                bt = b0 // 128

                # 1/denom broadcast to all partitions, [128, gb]
                rbp = psw.tile([128, gb], F32, tag="w")
                nc.tensor.matmul(
                    rbp, ones_row, recipr[0:1, b0 : b0 + gb], start=True, stop=True
                )
                rb = grp.tile([128, gb], F32, tag="rb")
                nc.vector.tensor_copy(rb, rbp)

                # attention apply: attnT[:, dt, b] += E[b,lt]^T[:,dt] @ wexp[b,lt]
                psAT = psa.tile([128, HT, gb], F32, tag="attn")
                for c in range(n_chunks):
                    cb0 = b0 + c * gcb
                    et0 = epool.tile([128, cb, H], F16, tag="E")
                    et1 = epool.tile([128, cb, H], F16, tag="E")
                    nc.gpsimd.dma_start(
                        out=et0[:, :gcb, :],
                        in_=enc_d[cb0 : cb0 + gcb, 0:128, :].rearrange(
                            "b l d -> l b d"
                        ),
                    )
                    nc.gpsimd.dma_start(
                        out=et1[:, :gcb, :],
                        in_=enc_d[cb0 : cb0 + gcb, 128:256, :].rearrange(
                            "b l d -> l b d"
                        ),
                    )
                    for bi in range(gcb):
                        b = cb0 + bi
                        bl = b - b0
                        for dt in range(HT):
                            nc.tensor.matmul(
                                psAT[:, dt, bl : bl + 1],
                                et0[:, bi, dt * 128 : (dt + 1) * 128],
                                wexpH[:, 0, b : b + 1],
                                start=True,
                                stop=False,
                            )
                            nc.tensor.matmul(
                                psAT[:, dt, bl : bl + 1],
                                et1[:, bi, dt * 128 : (dt + 1) * 128],
                                wexpH[:, 1, b : b + 1],
                                start=False,
                                stop=True,
                            )

                attnT = grp.tile([128, HT, gb], F16, tag="attnT")
                nc.vector.tensor_copy(attnT, psAT)

                # comb + relu -> reluT [h', b]
                reluT = grp.tile([128, HT, gb], F16, tag="reluT")
                for mt in range(HT):
                    cps = psw.tile([128, gb], F32, tag="w")
                    nc.tensor.matmul(
                        cps,
                        c12[:, mt * 128 : (mt + 1) * 128],
                        uv2[:, b0 : b0 + gb],
                        start=True,
                        stop=False,
                    )
                    for kt in range(HT):
                        nc.tensor.matmul(
                            cps,
                            w_comb[:, kt, mt * 128 : (mt + 1) * 128],
                            attnT[:, kt, :],
                            start=False,
                            stop=(kt == HT - 1),
                        )
                    # psum * (1/denom broadcast), then relu(. + comb_b)
                    stmp = ew.tile([128, gb], F32, tag="e")
                    nc.vector.tensor_mul(stmp, cps, rb)
                    nc.scalar.activation(
                        reluT[:, mt, :], stmp, AF.Relu, bias=combb[:, mt : mt + 1]
                    )

                # GRU gate matmuls
                rza = psw.tile([128, 512], F32, tag="w")
                rzb = psw.tile([128, 512], F32, tag="w")
                gin = psw.tile([128, 512], F32, tag="w")
                ghn = psw.tile([128, 512], F32, tag="w")
                for j in range(HT):
                    for dst, col0 in ((rza, j * 128), (rzb, 512 + j * 128)):
                        for kt in range(HT):
                            nc.tensor.matmul(
                                dst[:, j * 128 : j * 128 + gb],
                                w_ih[:, kt, col0 : col0 + 128],
                                reluT[:, kt, :],
                                start=(kt == 0),
                                stop=False,
                            )
                        for kt in range(HT):
                            nc.tensor.matmul(
                                dst[:, j * 128 : j * 128 + gb],
                                w_hh[:, kt, col0 : col0 + 128],
                                hidT_r[:, kt, b0 : b0 + gb],
                                start=False,
                                stop=(kt == HT - 1),
                            )
                    for dst, wsrc, rhs_fn in (
                        (gin, w_ih, lambda kt: reluT[:, kt, :]),
                        (ghn, w_hh, lambda kt: hidT_r[:, kt, b0 : b0 + gb]),
                    ):
                        col0 = 1024 + j * 128
                        for kt in range(HT):
                            nc.tensor.matmul(
                                dst[:, j * 128 : j * 128 + gb],
                                wsrc[:, kt, col0 : col0 + 128],
                                rhs_fn(kt),
                                start=(kt == 0),
                                stop=(kt == HT - 1),
                            )

                # elementwise GRU + outputs
                hnat = grp.tile([128, H], F32, tag="hnewnat")
                awn = grp.tile([128, L], F32, tag="awn")
                hnew_tiles = []
                for j in range(HT):
                    jc = slice(j * 128, j * 128 + gb)
                    r_j = ew.tile([128, gb], F32, tag="e")
                    nc.scalar.activation(
                        r_j, rza[:, jc], AF.Sigmoid, bias=brz[:, j : j + 1]
                    )
                    z_j = ew.tile([128, gb], F32, tag="e")
                    nc.scalar.activation(
                        z_j, rzb[:, jc], AF.Sigmoid, bias=brz[:, HT + j : HT + j + 1]
                    )
                    s1 = ew.tile([128, gb], F32, tag="e")
                    nc.vector.scalar_tensor_tensor(
                        s1, ghn[:, jc], bhhn[:, j : j + 1], r_j,
                        op0=ALU.add, op1=ALU.mult,
                    )
                    s2 = ew.tile([128, gb], F32, tag="e")
                    nc.vector.tensor_add(s2, s1, gin[:, jc])
                    n_j = ew.tile([128, gb], F32, tag="e")
                    nc.scalar.activation(n_j, s2, AF.Tanh, bias=bihn[:, j : j + 1])
                    d_j = ew.tile([128, gb], F32, tag="e")
                    nc.vector.tensor_sub(d_j, hidT[:, j, b0 : b0 + gb], n_j)
                    m_j = ew.tile([128, gb], F32, tag="e")
                    nc.vector.tensor_mul(m_j, z_j, d_j)
                    hnew_j = ew.tile([128, gb], F32, tag="hn")
                    nc.vector.tensor_add(hnew_j, n_j, m_j)
                    hnew_tiles.append(hnew_j)
                    tp = psw.tile([128, 128], F32, tag="w")
                    nc.tensor.transpose(tp[:gb, :], hnew_j, ident[:, :])
                    nc.vector.tensor_copy(
                        hnat[:gb, j * 128 : (j + 1) * 128], tp[:gb, :]
                    )
                nc.sync.dma_start(out=oh_d[b0 : b0 + gb, :], in_=hnat[:gb, :])

                # final output scalar: dot(h_new, out_W) + out_b
                oy = psw.tile([1, gb], F32, tag="w")
                for kt in range(HT):
                    nc.tensor.matmul(
                        oy,
                        outw[:, kt : kt + 1],
                        hnew_tiles[kt],
                        start=(kt == 0),
                        stop=(kt == HT - 1),
                    )
                oys = ew.tile([1, gb], F32, tag="oys")
                nc.scalar.activation(oys, oy, AF.Identity, bias=outb[0:1, 0:1])
                nc.sync.dma_start(
                    out=oy_d[b0 : b0 + gb].rearrange("(p f) -> p f", p=1), in_=oys
                )

                # normalized attention weights, natural layout
                for lt in range(LT):
                    tp = psw.tile([128, 128], F32, tag="w")
                    nc.tensor.transpose(
                        tp[:gb, :], wexpT[:, lt, b0 : b0 + gb], ident[:, :]
                    )
                    nc.vector.tensor_scalar(
                        awn[:gb, lt * 128 : (lt + 1) * 128],
                        tp[:gb, :],
                        recipc[b0 - bt * 128 : b0 - bt * 128 + gb, bt : bt + 1],
                        None,
                        op0=ALU.mult,
                    )
                nc.sync.dma_start(out=oaw_d[b0 : b0 + gb, :], in_=awn[:gb, :])
                b0 += gb

    return nc


def host_constants(emb_W, emb_b, attn_W, attn_b, comb_W, comb_b, gru_bih, gru_bhh,
                   out_W, out_b):
    """Fold weight-only constants (float64 for accuracy, cast to f32)."""
    W = emb_W.astype(np.float64).reshape(H)
    bb = emb_b.astype(np.float64)
    Wc = W - W.mean()
    bc = bb - bb.mean()
    a2 = float((Wc * Wc).mean())
    a1 = float(2.0 * (Wc * bc).mean())
    a0 = float((bc * bc).mean())
    attn_W1 = attn_W[:H].astype(np.float64)
    attn_W2 = attn_W[H:].astype(np.float32)
    comb_W1 = comb_W[:H].astype(np.float64)
    comb_W2 = comb_W[H:].astype(np.float32)
    p12 = np.stack([Wc @ attn_W1, bc @ attn_W1]).astype(np.float32)
    c12 = np.stack([Wc @ comb_W1, bc @ comb_W1]).astype(np.float32)
    coef = np.array([a2, a1, a0 + EPS], np.float32)
    brz = (gru_bih.astype(np.float64) + gru_bhh.astype(np.float64))[: 2 * H].astype(
        np.float32
    )
    sm = np.zeros((128, 928), np.float32)
    sm[:, 0:2] = attn_b.astype(np.float32).reshape(2, 128).T
    sm[:, 2:6] = comb_b.astype(np.float32).reshape(4, 128).T
    sm[:, 6:14] = brz.reshape(8, 128).T
    sm[:, 14:18] = gru_bih[2 * H :].astype(np.float32).reshape(4, 128).T
    sm[:, 18:22] = gru_bhh[2 * H :].astype(np.float32).reshape(4, 128).T
    sm[:, 22:26] = out_W[:, 0].astype(np.float32).reshape(4, 128).T
    # cols 26, 27:155 are memset to 1.0 on device (ones columns)
    sm[0:2, 27:283] = p12
    sm[0:2, 283:795] = c12
    sm[0:1, 795:798] = coef
    sm[0, 798] = float(out_b[0])
    return dict(
        wattn=np.ascontiguousarray(attn_W2),
        wcomb=np.ascontiguousarray(comb_W2.astype(np.float16)),
        smalls=sm,
        ident=np.eye(128, dtype=np.float32),
    )


def _split_waits(nc):
    """Walrus codegen in this toolchain gives most ISA instructions a single
    sync-wait slot (DMA triggers fit two) and refuses to split multi-wait
    instructions itself. Insert same-engine NoOps, each absorbing one excess
    wait, ahead of any over-subscribed instruction."""
    for f in nc.m.functions:
        for blk in f.blocks:
            out = []
            changed = False
            for inst in blk.instructions:
                si = inst.sync_info
                limit = 1
                if si is not None and len(si.on_wait) > limit:
                    waits = list(si.on_wait)
                    for w in waits[:-limit]:
                        nop = mybir.InstNoOp(
                            name=nc.get_next_instruction_name(),
                            engine=inst.engine,
                        )
                        nop.sync_info = mybir.SyncInfo(on_wait=[w], on_update=[])
                        out.append(nop)
                    inst.sync_info = mybir.SyncInfo(
                        on_wait=waits[-limit:], on_update=list(si.on_update)
                    )
                    changed = True
                out.append(inst)
            if changed:
                blk.instructions = out
    return nc


_PROGRAM_CACHE = {}
TRACE = False  # set by test harness to capture an NTFF profile
LAST_RESULTS = None


def _get_program(key=(BC, None, 8)):
    if key not in _PROGRAM_CACHE:
        _PROGRAM_CACHE[key] = _split_waits(build_program(*key))
    return _PROGRAM_CACHE[key]


def kernel(x, hidden, encoder_outputs, emb_W, emb_b, attn_W, attn_b,
           comb_W, comb_b, gru_Wih, gru_Whh, gru_bih, gru_bhh, out_W, out_b):
    from concourse.bass_utils import run_bass_kernel_spmd

    x = np.asarray(x, np.float32)
    hidden = np.asarray(hidden, np.float32)
    encoder_outputs = np.asarray(encoder_outputs, np.float32)

    const = host_constants(
        np.asarray(emb_W), np.asarray(emb_b), np.asarray(attn_W),
        np.asarray(attn_b), np.asarray(comb_W), np.asarray(comb_b),
        np.asarray(gru_bih), np.asarray(gru_bhh), np.asarray(out_W),
        np.asarray(out_b),
    )
    const["wih"] = np.ascontiguousarray(np.asarray(gru_Wih, np.float16))
    const["whh"] = np.ascontiguousarray(np.asarray(gru_Whh, np.float16))

    xf = x.reshape(B)
    hid = hidden.reshape(B, H)

    in_maps = []
    for c in range(NCORES):
        sl = slice(c * BC, (c + 1) * BC)
        m = dict(const)
        m["x"] = np.ascontiguousarray(xf[sl])
        m["hid"] = np.ascontiguousarray(hid[sl])
        m["enc"] = np.ascontiguousarray(encoder_outputs[sl])
        in_maps.append(m)

    nc = _get_program()
    res = run_bass_kernel_spmd(
        nc, in_maps, core_ids=list(range(NCORES)), trace=TRACE
    )
    globals()["LAST_RESULTS"] = res

    out_y = np.concatenate([r["out_y"] for r in res.results]).reshape(B, 1, 1)
    out_h = np.concatenate([r["out_h"] for r in res.results]).reshape(1, B, H)
    out_aw = np.concatenate([r["out_aw"] for r in res.results]).reshape(B, 1, L)
    return out_y, out_h, out_aw
